# revision 14
# baseline (speedup 1.0000x reference)
"""CLUB loss kernel for Trainium2, 8 NeuronCores, data-parallel over batch.

v2: fp8-e4m3 DoubleRow matmuls (2 contraction rows/PE-cell/cycle) halve the
PE time of the four 768-deep matmuls vs the bf16 baseline.  Tolerance is
2e-2 relative on two scalars that average ~6.3M elementwise terms, so fp8
quantization noise (~3% per weight/activation) washes out to ~1e-4.

Math (see reference): two MLPs over modal_a produce mu and logvar; the loss
needs only per-feature batch sums.  Let m0 = the second linear's output
WITHOUT the b2m bias (bias folded out on the host), iv = exp(-logvar),
mi = m0*iv, b' = modal_b - b2m:
  S1' = sum m0        S2' = sum m0^2      T0 = sum iv
  T1' = sum mi        T2' = sum m0*mi
  Pa  = sum b'*mi     Pb  = sum b'^2*iv
Then P = sum (mu-b)^2 iv = T2' - 2 Pa + Pb, and the b2m corrections for
S1/S2/T1/T2 are applied on the host (cheap [768]-vector algebra in f64).
This keeps the entire device epilogue to 4 ACT ops (3 tanh + 1 exp with a
free T0 accumulation) and 6 DVE ops per feature tile -- all below the PE
floor, so the kernel is matmul-bound.

Quantization scales (powers of two, exact): weights *2^12 (uniform +/-
1/sqrt(768) -> +/-148, inside fp8e4m3's +/-240 normal range), modal_a *2^5;
tanh outputs are written as raw fp8 (in (-1,1), mostly normal range).  The
inverse scales fold into the free affine input of the ACT/DVE epilogues.

Device layout: [feature, batch], all operands host-pretransposed; biases
fuse into ScalarE activations; batch sums are free-dim accumulations fused
into the same instructions (accum_out).  Weights stay stationary for both
512-wide moving chunks (kk-outer / ic-inner) and the duplicate LDWEIGHTS
is stripped post-finalize, so LDW (213ns) hides under 2x241ns of matmul.
"""

import numpy as np
import ml_dtypes

import concourse.bacc as bacc
import concourse.tile as tile
import concourse.mybir as mybir
from concourse.bass_utils import run_bass_kernel_spmd

B, H = 8192, 768
NCORES = 8
BS = B // NCORES          # 1024 rows per core
P = 128
KT = H // P               # 6 contraction tiles of 128
KK = KT // 2              # 3 DoubleRow contraction pairs (256 each)
JT = H // P               # 6 output-feature tiles
NI = 2                    # 512-wide moving chunks per matmul
IC = BS // NI             # 512

F32 = mybir.dt.float32
BF16 = mybir.dt.bfloat16
FP8 = mybir.dt.float8e4
AF = mybir.ActivationFunctionType
ALU = mybir.AluOpType
DR = mybir.MatmulPerfMode.DoubleRow

_BF16 = ml_dtypes.bfloat16
_FP8 = ml_dtypes.float8_e4m3

SW = 2.0 ** 12            # weight scale
SA = 2.0 ** 5             # modal_a scale
SC1 = 1.0 / (SW * SA)     # layer-1 psum descale
SC2 = 1.0 / SW            # layer-2 psum descale

_CACHE = {}
# Counter-intuitive but measured: keeping one LDWEIGHTS per matmul is ~11us
# FASTER than deduping the second load of a reused weight pair.  With
# LDW,MM,MM the in-flight second MM blocks the next LDW's silicon pull-ahead
# (it would clobber the foreground weights), serializing LDW; with one LDW
# per MM the loads pipeline into the background buffer (~271ns/pair).
DEDUP_LDW = False
SW_INTERLEAVE = False
PROBE_NO_STATS = False
# m0 (psum->sbuf convert + S1' accum) runs on ScalarE: ACT sits next to
# PSUM and this keeps the DVE chain off the PSUM port (measured -5.2us).
M0_ON_ACT = True
DMA_ON_SYNC = False
DRX = mybir.MatmulPerfMode.DoubleRowSwInterleave


def _build(repeat=1, trace_sim=False, loop_n=None):
    nc = bacc.Bacc(trn_type="TRN2")

    aT_d = nc.dram_tensor("aT", [H, BS], FP8, kind="ExternalInput")
    bT_d = nc.dram_tensor("bT", [H, BS], BF16, kind="ExternalInput")
    b2T_d = nc.dram_tensor("b2T", [H, BS], BF16, kind="ExternalInput")
    w_d = {
        name: nc.dram_tensor(name, [H, H], FP8, kind="ExternalInput")
        for name in ("w1vT", "w2vT", "w1mT", "w2mT")
    }
    bias_d = nc.dram_tensor("biases", [P, 3 * JT], F32, kind="ExternalInput")
    stats_d = nc.dram_tensor("stats", [H, 8], F32, kind="ExternalOutput")

    with tile.TileContext(nc, trace_sim=trace_sim) as tc:
        with (
            tc.tile_pool(name="weights", bufs=1) as wp,
            tc.tile_pool(name="acts", bufs=1) as ap,
            tc.tile_pool(name="rot", bufs=3) as rot,
            tc.tile_pool(name="stat", bufs=3) as stp,
            tc.tile_pool(name="psum", bufs=4, space="PSUM") as pp,
        ):
            # --- persistent SBUF tensors -------------------------------------
            w_sb = {}
            for name in ("w1vT", "w1mT", "w2vT", "w2mT"):
                w_sb[name] = wp.tile([P, KT, H], FP8, name=f"{name}_sb")
            aT_sb = ap.tile([P, KT, BS], FP8, name="aT_sb")
            bT_sb = ap.tile([P, JT, BS], BF16, name="bT_sb")
            b2T_sb = ap.tile([P, JT, BS], BF16, name="b2T_sb")
            h1v_sb = ap.tile([P, JT, BS], FP8, name="h1v_sb")
            h1m_sb = ap.tile([P, JT, BS], FP8, name="h1m_sb")
            bias_sb = ap.tile([P, 3 * JT], F32, name="bias_sb")

            # --- input DMAs (phase order: first matmul's operands first) -----
            nc.sync.dma_start(bias_sb, bias_d[:, :])
            for kt in range(KT):
                nc.sync.dma_start(
                    w_sb["w1vT"][:, kt, :], w_d["w1vT"][kt * P:(kt + 1) * P, :])
                nc.sync.dma_start(
                    aT_sb[:, kt, :], aT_d[kt * P:(kt + 1) * P, :])
            for kt in range(KT):
                nc.sync.dma_start(
                    w_sb["w1mT"][:, kt, :], w_d["w1mT"][kt * P:(kt + 1) * P, :])
            for kt in range(KT):
                nc.sync.dma_start(
                    w_sb["w2vT"][:, kt, :], w_d["w2vT"][kt * P:(kt + 1) * P, :])
            for kt in range(KT):
                nc.sync.dma_start(
                    w_sb["w2mT"][:, kt, :], w_d["w2mT"][kt * P:(kt + 1) * P, :])
            for kt in range(KT):
                nc.sync.dma_start(
                    bT_sb[:, kt, :], bT_d[kt * P:(kt + 1) * P, :])
            for kt in range(KT):
                nc.sync.dma_start(
                    b2T_sb[:, kt, :], b2T_d[kt * P:(kt + 1) * P, :])

            def matmul_dr(w, rhs_sb, j):
                """768-deep fp8 DoubleRow matmul for feature tile j.

                kk-outer / ic-inner: each 256-row weight pair is loaded once
                and streams both 512-wide moving chunks (the duplicate
                LDWEIGHTS is stripped by _dedup_ldweights)."""
                ps = pp.tile([P, BS], F32, tag="ps", name="ps")
                for kk in range(KK):
                    if SW_INTERLEAVE:
                        # timing probe: contiguous 256-col weight slice
                        lhsT = w[:, 2 * kk, j * P:j * P + 2 * P]
                        pm = DRX
                    else:
                        lhsT = w[:, 2 * kk:2 * kk + 2, j * P:(j + 1) * P]
                        pm = DR
                    for ic in range(NI):
                        nc.tensor.matmul(
                            ps[:, ic * IC:(ic + 1) * IC], lhsT,
                            rhs_sb[:, 2 * kk:2 * kk + 2, ic * IC:(ic + 1) * IC],
                            start=(kk == 0), stop=(kk == KK - 1),
                            perf_mode=pm)
                return ps

            def l1_tile(wname, out_sb, j, bias_col):
                ps = matmul_dr(w_sb[wname], aT_sb, j)
                nc.scalar.activation(
                    out_sb[:, j, :], ps, AF.Tanh,
                    bias=bias_sb[:, bias_col:bias_col + 1], scale=SC1)

            def l2v_tile(jv):
                """lv = tanh(psum/SW + b2v); iv = exp(-lv), T0 accumulated."""
                ps = matmul_dr(w_sb["w2vT"], h1v_sb, jv)
                if PROBE_NO_STATS:
                    return None
                lv = rot.tile([P, BS], BF16, tag="lv")
                nc.scalar.activation(
                    lv, ps, AF.Tanh,
                    bias=bias_sb[:, 1 * JT + jv:1 * JT + jv + 1], scale=SC2)
                sa = stp.tile([P, 1], F32, tag="st_a")
                iv = rot.tile([P, BS], BF16, tag="iv")
                nc.scalar.activation(iv, lv, AF.Exp, scale=-1.0, accum_out=sa)
                (nc.sync if DMA_ON_SYNC else nc.gpsimd).dma_start(
                    stats_d[jv * P:(jv + 1) * P, 6:7], sa)
                return iv

            def l2m_tile(j, iv):
                """m0 = psum/SW (no bias) and all six fused statistics."""
                ps2 = matmul_dr(w_sb["w2mT"], h1m_sb, j)
                if PROBE_NO_STATS:
                    return
                sv = stp.tile([P, 6], F32, tag="st_v")
                m0 = rot.tile([P, BS], BF16, tag="m0")
                if M0_ON_ACT:
                    # S1' = sum m0 (ACT sits next to PSUM; frees a DVE slot)
                    sa2 = stp.tile([P, 1], F32, tag="st_a2")
                    nc.scalar.activation(
                        m0, ps2, AF.Identity, scale=SC2, accum_out=sa2)
                    (nc.sync if DMA_ON_SYNC else nc.gpsimd).dma_start(
                        stats_d[j * P:(j + 1) * P, 7:8], sa2)
                else:
                    # S1' = sum m0
                    nc.vector.tensor_scalar(
                        m0, ps2, SC2, 0.0, ALU.mult, ALU.add,
                        accum_out=sv[:, 0:1])
                # T1' = sum m0*iv (keeps mi)
                mi = rot.tile([P, BS], BF16, tag="mi")
                nc.vector.scalar_tensor_tensor(
                    mi, m0, 1.0, iv, ALU.bypass, ALU.mult,
                    accum_out=sv[:, 2:3])
                # T2' = sum m0*mi
                scr0 = rot.tile([P, BS], BF16, tag="scr0")
                nc.vector.scalar_tensor_tensor(
                    scr0, m0, 1.0, mi, ALU.bypass, ALU.mult,
                    accum_out=sv[:, 3:4])
                # S2' = sum m0*m0
                scr1 = rot.tile([P, BS], BF16, tag="scr1")
                nc.vector.scalar_tensor_tensor(
                    scr1, m0, 1.0, m0, ALU.bypass, ALU.mult,
                    accum_out=sv[:, 1:2])
                # Pa = sum b'*mi
                scr2 = rot.tile([P, BS], BF16, tag="scr2")
                nc.vector.scalar_tensor_tensor(
                    scr2, bT_sb[:, j, :], 1.0, mi, ALU.bypass, ALU.mult,
                    accum_out=sv[:, 4:5])
                # Pb = sum b'^2*iv
                scr3 = rot.tile([P, BS], BF16, tag="scr3")
                nc.vector.scalar_tensor_tensor(
                    scr3, b2T_sb[:, j, :], 1.0, iv, ALU.bypass, ALU.mult,
                    accum_out=sv[:, 5:6])
                (nc.sync if DMA_ON_SYNC else nc.gpsimd).dma_start(
                    stats_d[j * P:(j + 1) * P, 0:6], sv)

            def body():
                for j in range(JT):
                    l1_tile("w1vT", h1v_sb, j, 0 * JT + j)
                for j in range(JT):
                    l1_tile("w1mT", h1m_sb, j, 2 * JT + j)
                # interleaved L2, V one tile ahead of M
                ivs = [None] * JT
                ivs[0] = l2v_tile(0)
                for j in range(JT - 1):
                    ivs[j + 1] = l2v_tile(j + 1)
                    l2m_tile(j, ivs[j])
                l2m_tile(JT - 1, ivs[JT - 1])

            if loop_n is not None:
                with tc.For_i(0, loop_n, 1,
                              hint_engines=(mybir.EngineType.PE,
                                            mybir.EngineType.Activation,
                                            mybir.EngineType.DVE,
                                            mybir.EngineType.Pool)):
                    for _rep in range(repeat):
                        body()
            else:
                for _rep in range(repeat):
                    body()

    nc.finalize()
    if DEDUP_LDW:
        _dedup_ldweights(nc)
    return nc


def _dedup_ldweights(nc):
    """Drop InstLdweights whose weights AP is identical to the previous PE
    weight load with only matmuls in between -- the weights are still
    resident in the PE array (bass emits one load per matmul with no reuse
    detection)."""
    removed = 0
    for f in nc.m.functions:
        for bb in f.blocks:
            insts = list(bb.instructions)
            keep = []
            last_sig = None
            ok_since = True
            for ins in insts:
                eng = str(getattr(ins, "engine", ""))
                nm = type(ins).__name__
                if eng == "EngineType.PE":
                    if nm == "InstLdweights":
                        sig = str(ins.ins[0])
                        si = ins.sync_info
                        nw = len(si.on_wait) if si else 0
                        if sig == last_sig and ok_since and nw == 0:
                            removed += 1
                            continue
                        last_sig = sig
                        ok_since = True
                    elif nm != "InstMatmult":
                        ok_since = False
                        last_sig = None
                keep.append(ins)
            if len(keep) != len(insts):
                while len(bb.instructions):
                    bb.instructions.pop()
                for ins in keep:
                    bb.instructions.append(ins)
    return removed


def _q8(x, scale):
    return np.clip(np.asarray(x, np.float32) * scale, -240.0, 240.0).astype(_FP8)


def prepare_in_maps(modal_a, modal_b, W1m, b1m, W2m, b2m, W1v, b1v, W2v, b2v):
    w1mT = np.ascontiguousarray(_q8(W1m, SW).T)
    w2mT = np.ascontiguousarray(_q8(W2m, SW).T)
    w1vT = np.ascontiguousarray(_q8(W1v, SW).T)
    w2vT = np.ascontiguousarray(_q8(W2v, SW).T)
    bias_pack = np.zeros((P, 3 * JT), np.float32)
    for l, bias in enumerate((b1v, b2v, b1m)):
        bias_pack[:, l * JT:(l + 1) * JT] = np.asarray(
            bias, np.float32).reshape(JT, P).T

    a8 = _q8(modal_a, SA)
    bprime = np.asarray(modal_b, np.float32) - np.asarray(b2m, np.float32)[None, :]
    b_bf = bprime.astype(_BF16)
    b2_bf = (bprime * bprime).astype(_BF16)
    in_maps = []
    for c in range(NCORES):
        rows = slice(c * BS, (c + 1) * BS)
        in_maps.append({
            "aT": np.ascontiguousarray(a8[rows].T),
            "bT": np.ascontiguousarray(b_bf[rows].T),
            "b2T": np.ascontiguousarray(b2_bf[rows].T),
            "w1mT": w1mT, "w2mT": w2mT, "w1vT": w1vT, "w2vT": w2vT,
            "biases": bias_pack,
        })
    return in_maps


def combine_stats(stats_list, b2m):
    stats_list = list(stats_list)
    acc = np.zeros((H, 7), np.float64)
    for st in stats_list:
        acc += st[:, 0:7].astype(np.float64)
    S1p, S2p, T1p, T2p, Pa, Pb, T0 = [acc[:, i] for i in range(7)]
    if M0_ON_ACT:
        S1p = np.add.reduce([st[:, 7].astype(np.float64) for st in stats_list])
    b2 = np.asarray(b2m, np.float64)

    S1 = S1p + B * b2
    S2 = S2p + 2.0 * b2 * S1p + B * b2 * b2
    T1 = T1p + b2 * T0
    T2 = T2p + 2.0 * b2 * T1p + b2 * b2 * T0
    Ptot = (T2p - 2.0 * Pa + Pb).sum()

    mu_mean = S1 / B
    mu_sq_mean = S2 / B
    lld = -0.5 / B * Ptot
    neg_total = -0.5 * (mu_sq_mean @ T0 - 2.0 * (mu_mean @ T1) + T2.sum())
    bound = lld - neg_total / B
    return (np.float32(lld), np.float32(bound))


def kernel(modal_a, modal_b, W1m, b1m, W2m, b2m, W1v, b1v, W2v, b2v):
    if "nc" not in _CACHE:
        _CACHE["nc"] = _build()
    nc = _CACHE["nc"]

    in_maps = prepare_in_maps(modal_a, modal_b, W1m, b1m, W2m, b2m,
                              W1v, b1v, W2v, b2v)
    # One retry: a previously-wedged device surfaces as a runtime error on
    # the first execution and is reset by the failed attempt.
    try:
        res = run_bass_kernel_spmd(nc, in_maps, core_ids=list(range(NCORES)))
    except Exception:
        res = run_bass_kernel_spmd(nc, in_maps, core_ids=list(range(NCORES)))
    return combine_stats([res.results[c]["stats"] for c in range(NCORES)], b2m)


# revision 17
# speedup vs baseline: 1.1938x; 1.1938x over previous
"""CLUB loss kernel for Trainium2, 8 NeuronCores, data-parallel over batch.

v2: fp8-e4m3 DoubleRow matmuls (2 contraction rows/PE-cell/cycle) halve the
PE time of the four 768-deep matmuls vs the bf16 baseline.  Tolerance is
2e-2 relative on two scalars that average ~6.3M elementwise terms, so fp8
quantization noise (~3% per weight/activation) washes out to ~1e-4.

Math (see reference): two MLPs over modal_a produce mu and logvar; the loss
needs only per-feature batch sums.  Let m0 = the second linear's output
WITHOUT the b2m bias (bias folded out on the host), iv = exp(-logvar),
mi = m0*iv, b' = modal_b - b2m:
  S1' = sum m0        S2' = sum m0^2      T0 = sum iv
  T1' = sum mi        T2' = sum m0*mi
  Pa  = sum b'*mi     Pb  = sum b'^2*iv
Then P = sum (mu-b)^2 iv = T2' - 2 Pa + Pb, and the b2m corrections for
S1/S2/T1/T2 are applied on the host (cheap [768]-vector algebra in f64).
This keeps the entire device epilogue to 4 ACT ops (3 tanh + 1 exp with a
free T0 accumulation) and 6 DVE ops per feature tile -- all below the PE
floor, so the kernel is matmul-bound.

Quantization scales (powers of two, exact): weights *2^12 (uniform +/-
1/sqrt(768) -> +/-148, inside fp8e4m3's +/-240 normal range), modal_a *2^5;
tanh outputs are written as raw fp8 (in (-1,1), mostly normal range).  The
inverse scales fold into the free affine input of the ACT/DVE epilogues.

Device layout: [feature, batch], all operands host-pretransposed; biases
fuse into ScalarE activations; batch sums are free-dim accumulations fused
into the same instructions (accum_out).  One LDWEIGHTS per matmul pipelines
the 256-column weight loads into the PE background buffer under the
in-flight matmul (measured faster than deduplicating reused loads).
"""

import numpy as np
import ml_dtypes

import concourse.bacc as bacc
import concourse.tile as tile
import concourse.mybir as mybir
from concourse.bass_utils import run_bass_kernel_spmd

B, H = 8192, 768
NCORES = 8
BS = B // NCORES          # 1024 rows per core
P = 128
KT = H // P               # 6 contraction tiles of 128
KK = KT // 2              # 3 DoubleRow contraction pairs (256 each)
JT = H // P               # 6 output-feature tiles
NI = 2                    # 512-wide moving chunks per matmul
IC = BS // NI             # 512

F32 = mybir.dt.float32
BF16 = mybir.dt.bfloat16
FP8 = mybir.dt.float8e4
AF = mybir.ActivationFunctionType
ALU = mybir.AluOpType
DR = mybir.MatmulPerfMode.DoubleRow

_BF16 = ml_dtypes.bfloat16
_FP8 = ml_dtypes.float8_e4m3

SW = 2.0 ** 12            # weight scale
SA = 2.0 ** 5             # modal_a scale
SC1 = 1.0 / (SW * SA)     # layer-1 psum descale
SC2 = 1.0 / SW            # layer-2 psum descale

_CACHE = {}
# Counter-intuitive but measured: keeping one LDWEIGHTS per matmul is ~11us
# FASTER than deduping the second load of a reused weight pair.  With
# LDW,MM,MM the in-flight second MM blocks the next LDW's silicon pull-ahead
# (it would clobber the foreground weights), serializing LDW; with one LDW
# per MM the loads pipeline into the background buffer (~271ns/pair).
DEDUP_LDW = False
SW_INTERLEAVE = False
PROBE_NO_STATS = False
# m0 (psum->sbuf convert + S1' accum) runs on ScalarE: ACT sits next to
# PSUM and this keeps the DVE chain off the PSUM port (measured -5.2us).
M0_ON_ACT = True
DMA_ON_SYNC = False
L1_INTERLEAVE = True
V2_AHEAD = True
ROT_BUFS = 3
DRX = mybir.MatmulPerfMode.DoubleRowSwInterleave


def _build(repeat=1, trace_sim=False, loop_n=None):
    nc = bacc.Bacc(trn_type="TRN2")

    aT_d = nc.dram_tensor("aT", [H, BS], FP8, kind="ExternalInput")
    bT_d = nc.dram_tensor("bT", [H, BS], BF16, kind="ExternalInput")
    b2T_d = nc.dram_tensor("b2T", [H, BS], BF16, kind="ExternalInput")
    w_d = {
        name: nc.dram_tensor(name, [H, H], FP8, kind="ExternalInput")
        for name in ("w1vT", "w2vT", "w1mT", "w2mT")
    }
    bias_d = nc.dram_tensor("biases", [P, 3 * JT], F32, kind="ExternalInput")
    stats_d = nc.dram_tensor("stats", [H, 8], F32, kind="ExternalOutput")

    with tile.TileContext(nc, trace_sim=trace_sim) as tc:
        with (
            tc.tile_pool(name="weights", bufs=1) as wp,
            tc.tile_pool(name="acts", bufs=1) as ap,
            tc.tile_pool(name="rot", bufs=ROT_BUFS) as rot,
            tc.tile_pool(name="stat", bufs=ROT_BUFS) as stp,
            tc.tile_pool(name="psum", bufs=4, space="PSUM") as pp,
        ):
            # --- persistent SBUF tensors -------------------------------------
            w_sb = {}
            for name in ("w1vT", "w1mT", "w2vT", "w2mT"):
                w_sb[name] = wp.tile([P, KT, H], FP8, name=f"{name}_sb")
            aT_sb = ap.tile([P, KT, BS], FP8, name="aT_sb")
            bT_sb = ap.tile([P, JT, BS], BF16, name="bT_sb")
            b2T_sb = ap.tile([P, JT, BS], BF16, name="b2T_sb")
            h1v_sb = ap.tile([P, JT, BS], FP8, name="h1v_sb")
            h1m_sb = ap.tile([P, JT, BS], FP8, name="h1m_sb")
            bias_sb = ap.tile([P, 3 * JT], F32, name="bias_sb")

            # --- input DMAs (phase order: first matmul's operands first) -----
            nc.sync.dma_start(bias_sb, bias_d[:, :])
            for kt in range(KT):
                nc.sync.dma_start(
                    w_sb["w1vT"][:, kt, :], w_d["w1vT"][kt * P:(kt + 1) * P, :])
                nc.sync.dma_start(
                    aT_sb[:, kt, :], aT_d[kt * P:(kt + 1) * P, :])
            for kt in range(KT):
                nc.sync.dma_start(
                    w_sb["w1mT"][:, kt, :], w_d["w1mT"][kt * P:(kt + 1) * P, :])
            for kt in range(KT):
                nc.sync.dma_start(
                    w_sb["w2vT"][:, kt, :], w_d["w2vT"][kt * P:(kt + 1) * P, :])
            for kt in range(KT):
                nc.sync.dma_start(
                    w_sb["w2mT"][:, kt, :], w_d["w2mT"][kt * P:(kt + 1) * P, :])
            for kt in range(KT):
                nc.sync.dma_start(
                    bT_sb[:, kt, :], bT_d[kt * P:(kt + 1) * P, :])
            for kt in range(KT):
                nc.sync.dma_start(
                    b2T_sb[:, kt, :], b2T_d[kt * P:(kt + 1) * P, :])

            def matmul_dr(w, rhs_sb, j):
                """768-deep fp8 DoubleRow matmul for feature tile j.

                kk-outer / ic-inner; one LDWEIGHTS per matmul (see
                DEDUP_LDW note above -- the loads pipeline into the
                background weight buffer under the in-flight matmul)."""
                ps = pp.tile([P, BS], F32, tag="ps", name="ps")
                for kk in range(KK):
                    if SW_INTERLEAVE:
                        # timing probe: contiguous 256-col weight slice
                        lhsT = w[:, 2 * kk, j * P:j * P + 2 * P]
                        pm = DRX
                    else:
                        lhsT = w[:, 2 * kk:2 * kk + 2, j * P:(j + 1) * P]
                        pm = DR
                    for ic in range(NI):
                        nc.tensor.matmul(
                            ps[:, ic * IC:(ic + 1) * IC], lhsT,
                            rhs_sb[:, 2 * kk:2 * kk + 2, ic * IC:(ic + 1) * IC],
                            start=(kk == 0), stop=(kk == KK - 1),
                            perf_mode=pm)
                return ps

            def l1_tile(wname, out_sb, j, bias_col):
                ps = matmul_dr(w_sb[wname], aT_sb, j)
                nc.scalar.activation(
                    out_sb[:, j, :], ps, AF.Tanh,
                    bias=bias_sb[:, bias_col:bias_col + 1], scale=SC1)

            def l2v_tile(jv):
                """lv = tanh(psum/SW + b2v); iv = exp(-lv), T0 accumulated."""
                ps = matmul_dr(w_sb["w2vT"], h1v_sb, jv)
                if PROBE_NO_STATS:
                    return None
                lv = rot.tile([P, BS], BF16, tag="lv")
                nc.scalar.activation(
                    lv, ps, AF.Tanh,
                    bias=bias_sb[:, 1 * JT + jv:1 * JT + jv + 1], scale=SC2)
                sa = stp.tile([P, 1], F32, tag="st_a")
                iv = rot.tile([P, BS], BF16, tag="iv")
                nc.scalar.activation(iv, lv, AF.Exp, scale=-1.0, accum_out=sa)
                (nc.sync if DMA_ON_SYNC else nc.gpsimd).dma_start(
                    stats_d[jv * P:(jv + 1) * P, 6:7], sa)
                return iv

            def l2m_tile(j, iv):
                """m0 = psum/SW (no bias) and all six fused statistics."""
                ps2 = matmul_dr(w_sb["w2mT"], h1m_sb, j)
                if PROBE_NO_STATS:
                    return
                sv = stp.tile([P, 6], F32, tag="st_v")
                m0 = rot.tile([P, BS], BF16, tag="m0")
                if M0_ON_ACT:
                    # S1' = sum m0 (ACT sits next to PSUM; frees a DVE slot)
                    sa2 = stp.tile([P, 1], F32, tag="st_a2")
                    nc.scalar.activation(
                        m0, ps2, AF.Identity, scale=SC2, accum_out=sa2)
                    (nc.sync if DMA_ON_SYNC else nc.gpsimd).dma_start(
                        stats_d[j * P:(j + 1) * P, 7:8], sa2)
                else:
                    # S1' = sum m0
                    nc.vector.tensor_scalar(
                        m0, ps2, SC2, 0.0, ALU.mult, ALU.add,
                        accum_out=sv[:, 0:1])
                # T1' = sum m0*iv (keeps mi)
                mi = rot.tile([P, BS], BF16, tag="mi")
                nc.vector.scalar_tensor_tensor(
                    mi, m0, 1.0, iv, ALU.bypass, ALU.mult,
                    accum_out=sv[:, 2:3])
                # T2' = sum m0*mi
                scr0 = rot.tile([P, BS], BF16, tag="scr0")
                nc.vector.scalar_tensor_tensor(
                    scr0, m0, 1.0, mi, ALU.bypass, ALU.mult,
                    accum_out=sv[:, 3:4])
                # S2' = sum m0*m0
                scr1 = rot.tile([P, BS], BF16, tag="scr1")
                nc.vector.scalar_tensor_tensor(
                    scr1, m0, 1.0, m0, ALU.bypass, ALU.mult,
                    accum_out=sv[:, 1:2])
                # Pa = sum b'*mi
                scr2 = rot.tile([P, BS], BF16, tag="scr2")
                nc.vector.scalar_tensor_tensor(
                    scr2, bT_sb[:, j, :], 1.0, mi, ALU.bypass, ALU.mult,
                    accum_out=sv[:, 4:5])
                # Pb = sum b'^2*iv
                scr3 = rot.tile([P, BS], BF16, tag="scr3")
                nc.vector.scalar_tensor_tensor(
                    scr3, b2T_sb[:, j, :], 1.0, iv, ALU.bypass, ALU.mult,
                    accum_out=sv[:, 5:6])
                (nc.sync if DMA_ON_SYNC else nc.gpsimd).dma_start(
                    stats_d[j * P:(j + 1) * P, 0:6], sv)

            def body():
                if L1_INTERLEAVE:
                    for j in range(JT):
                        l1_tile("w1vT", h1v_sb, j, 0 * JT + j)
                        l1_tile("w1mT", h1m_sb, j, 2 * JT + j)
                else:
                    for j in range(JT):
                        l1_tile("w1vT", h1v_sb, j, 0 * JT + j)
                    for j in range(JT):
                        l1_tile("w1mT", h1m_sb, j, 2 * JT + j)
                ivs = [None] * JT
                if V2_AHEAD:
                    # interleaved L2, V two tiles ahead of M
                    ivs[0] = l2v_tile(0)
                    ivs[1] = l2v_tile(1)
                    for j in range(JT - 2):
                        ivs[j + 2] = l2v_tile(j + 2)
                        l2m_tile(j, ivs[j])
                    l2m_tile(JT - 2, ivs[JT - 2])
                    l2m_tile(JT - 1, ivs[JT - 1])
                else:
                    # interleaved L2, V one tile ahead of M
                    ivs[0] = l2v_tile(0)
                    for j in range(JT - 1):
                        ivs[j + 1] = l2v_tile(j + 1)
                        l2m_tile(j, ivs[j])
                    l2m_tile(JT - 1, ivs[JT - 1])

            if loop_n is not None:
                with tc.For_i(0, loop_n, 1,
                              hint_engines=(mybir.EngineType.PE,
                                            mybir.EngineType.Activation,
                                            mybir.EngineType.DVE,
                                            mybir.EngineType.Pool)):
                    for _rep in range(repeat):
                        body()
            else:
                for _rep in range(repeat):
                    body()

    nc.finalize()
    if DEDUP_LDW:
        _dedup_ldweights(nc)
    return nc


def _dedup_ldweights(nc):
    """Drop InstLdweights whose weights AP is identical to the previous PE
    weight load with only matmuls in between -- the weights are still
    resident in the PE array (bass emits one load per matmul with no reuse
    detection)."""
    removed = 0
    for f in nc.m.functions:
        for bb in f.blocks:
            insts = list(bb.instructions)
            keep = []
            last_sig = None
            ok_since = True
            for ins in insts:
                eng = str(getattr(ins, "engine", ""))
                nm = type(ins).__name__
                if eng == "EngineType.PE":
                    if nm == "InstLdweights":
                        sig = str(ins.ins[0])
                        si = ins.sync_info
                        nw = len(si.on_wait) if si else 0
                        if sig == last_sig and ok_since and nw == 0:
                            removed += 1
                            continue
                        last_sig = sig
                        ok_since = True
                    elif nm != "InstMatmult":
                        ok_since = False
                        last_sig = None
                keep.append(ins)
            if len(keep) != len(insts):
                while len(bb.instructions):
                    bb.instructions.pop()
                for ins in keep:
                    bb.instructions.append(ins)
    return removed


def _q8(x, scale):
    return np.clip(np.asarray(x, np.float32) * scale, -240.0, 240.0).astype(_FP8)


def prepare_in_maps(modal_a, modal_b, W1m, b1m, W2m, b2m, W1v, b1v, W2v, b2v):
    w1mT = np.ascontiguousarray(_q8(W1m, SW).T)
    w2mT = np.ascontiguousarray(_q8(W2m, SW).T)
    w1vT = np.ascontiguousarray(_q8(W1v, SW).T)
    w2vT = np.ascontiguousarray(_q8(W2v, SW).T)
    bias_pack = np.zeros((P, 3 * JT), np.float32)
    for l, bias in enumerate((b1v, b2v, b1m)):
        bias_pack[:, l * JT:(l + 1) * JT] = np.asarray(
            bias, np.float32).reshape(JT, P).T

    a8 = _q8(modal_a, SA)
    bprime = np.asarray(modal_b, np.float32) - np.asarray(b2m, np.float32)[None, :]
    b_bf = bprime.astype(_BF16)
    b2_bf = (bprime * bprime).astype(_BF16)
    in_maps = []
    for c in range(NCORES):
        rows = slice(c * BS, (c + 1) * BS)
        in_maps.append({
            "aT": np.ascontiguousarray(a8[rows].T),
            "bT": np.ascontiguousarray(b_bf[rows].T),
            "b2T": np.ascontiguousarray(b2_bf[rows].T),
            "w1mT": w1mT, "w2mT": w2mT, "w1vT": w1vT, "w2vT": w2vT,
            "biases": bias_pack,
        })
    return in_maps


def combine_stats(stats_list, b2m):
    stats_list = list(stats_list)
    acc = np.zeros((H, 7), np.float64)
    for st in stats_list:
        acc += st[:, 0:7].astype(np.float64)
    S1p, S2p, T1p, T2p, Pa, Pb, T0 = [acc[:, i] for i in range(7)]
    if M0_ON_ACT:
        S1p = np.add.reduce([st[:, 7].astype(np.float64) for st in stats_list])
    b2 = np.asarray(b2m, np.float64)

    S1 = S1p + B * b2
    S2 = S2p + 2.0 * b2 * S1p + B * b2 * b2
    T1 = T1p + b2 * T0
    T2 = T2p + 2.0 * b2 * T1p + b2 * b2 * T0
    Ptot = (T2p - 2.0 * Pa + Pb).sum()

    mu_mean = S1 / B
    mu_sq_mean = S2 / B
    lld = -0.5 / B * Ptot
    neg_total = -0.5 * (mu_sq_mean @ T0 - 2.0 * (mu_mean @ T1) + T2.sum())
    bound = lld - neg_total / B
    return (np.float32(lld), np.float32(bound))


def kernel(modal_a, modal_b, W1m, b1m, W2m, b2m, W1v, b1v, W2v, b2v):
    if "nc" not in _CACHE:
        _CACHE["nc"] = _build()
    nc = _CACHE["nc"]

    in_maps = prepare_in_maps(modal_a, modal_b, W1m, b1m, W2m, b2m,
                              W1v, b1v, W2v, b2v)
    # One retry: a previously-wedged device surfaces as a runtime error on
    # the first execution and is reset by the failed attempt.
    try:
        res = run_bass_kernel_spmd(nc, in_maps, core_ids=list(range(NCORES)))
    except Exception:
        res = run_bass_kernel_spmd(nc, in_maps, core_ids=list(range(NCORES)))
    return combine_stats([res.results[c]["stats"] for c in range(NCORES)], b2m)
